# revision 1
# baseline (speedup 1.0000x reference)
"""Constraint-projection layer on 8 Trainium2 NeuronCores.

Reference computes, per batch row y_i:  x_i = argmin ||x - y_i|| s.t. A x = b_i
via a dense KKT solve. Closed form (Schur complement of the KKT system):

    x = y - A^T (A A^T)^{-1} (A y - b)

Host precomputes W = (A A^T)^{-1} A  (128 x 1024, float64 solve, cast f32).
Each core gets a 2048-row batch shard in TRANSPOSED layout (dim-major), so
both matmuls contract over the partition axis with contiguous DMA only:

    stage 1:  T^T = A @ Y^T - B^T          (128 m  x 2048 batch)
    stage 2:  X^T = Y^T - W_chunk^T @ T^T  (1024 d x 2048 batch)

Data-parallel: no cross-core communication.
"""

import os

import numpy as np
import bass_rust as _br
import concourse.bass as bass
import concourse.mybir as mybir
from concourse import tile
from concourse.bass_utils import run_bass_kernel_spmd

F32 = mybir.dt.float32
F32R = mybir.dt.float32r
# fp32r streams through the PE at 4x the fp32 rate (1 cycle/row vs 4), at
# slightly reduced multiply precision. Per-stage choice: stage 1 (the long
# 1024-term contraction) and stage 2 (the 128-term correction).
# Modes: "f32", "f32r", "hybrid1" (stage1 f32r), "hybrid2" (stage2 f32r).
# Default hybrid2: stage 1 (1024-term dots, dominates rounding error) stays
# f32; stage 2's short 128-term correction uses f32r. Measured absmax rel err
# 6.9e-5 vs 4.4e-5 all-f32, and the PE work fits under the DMA roofline.
MM_MODE = os.environ.get("KERNEL_MM_MODE", "hybrid2")
_S1_R = MM_MODE in ("f32r", "hybrid1")
_S2_R = MM_MODE in ("f32r", "hybrid2")


def _s1(ap):
    return ap.bitcast(F32R) if _S1_R else ap


def _s2(ap):
    return ap.bitcast(F32R) if _S2_R else ap

N_CORES = 8
BATCH = 16384
N = 1024           # input dim
M = 128            # constraint dim
BC = BATCH // N_CORES  # 2048 batch rows per core
KC = N // 128      # 8 contraction chunks
F = 512            # free-dim tile (one PSUM bank of f32)
NJ = BC // F       # 4 batch tiles per core


def _split_drain_and_barrier(self, tick_clock, wait_clock):
    # Walrus in this toolchain rejects >2 sync waits on the Tile tail Drain
    # (CTRL_NO_STRUCT). Emit one-wait-per-nop instructions ahead of the
    # drain instead; sequentially identical on the sync sequencer.
    gc = tick_clock.global_clock
    vals = eval(repr(gc).replace("VectorClock", "").strip("()"))
    for i, v in enumerate(vals):
        if v:
            single = [0] * len(vals)
            single[i] = v
            nop = self.nc.sync.nop(nofuse=True)
            wait_clock.add_sem_waits(
                nop.ins, _br.ScopedClock({None: _br.VectorClock(single)})
            )
    self.nc.sync.drain()
    self.nc.all_engine_barrier()
    assert self.sems is not None
    popped = self.nc._tile_sem_poison_stack.pop()
    assert popped is self._sem_poison
    self.nc.clear_and_free_semaphores(list(self.sems.allocated().values()))
    self.nc.all_engine_barrier()


tile.TileContext._drain_and_barrier = _split_drain_and_barrier

_orig_commit_and_lower = tile.TileContext._commit_and_lower

# Same walrus limitation for regular instructions: Matmult (S3_LW) takes no
# extra sync waits, most others take one. Spill excess waits onto dedicated
# same-engine nops committed immediately before the instruction.
_ZERO_WAIT_OPS = ("InstMatmult", "InstDrain")


def _split_commit_and_lower(self, inst, original_block, old_bb_map, bb_to_exit_bb):
    tn = type(inst).__name__
    if tn.startswith("Inst") and inst.engine is not None:
        si = inst.sync_info
        if si is not None:
            waits = list(si.on_wait)
            keep = 0 if tn in _ZERO_WAIT_OPS else 1
            if len(waits) > keep:
                spill, keep_waits = (
                    (waits, []) if keep == 0 else (waits[:-1], [waits[-1]])
                )
                for w_ in spill:
                    nop = mybir.InstNoOp(
                        name=self.nc.get_next_instruction_name(),
                        engine=inst.engine,
                        sync_info=mybir.SyncInfo(on_wait=[w_], on_update=[]),
                        bass_nofuse=True,
                    )
                    self._commit_instruction(nop)
                inst.sync_info = mybir.SyncInfo(
                    on_wait=keep_waits, on_update=list(si.on_update)
                )
    return _orig_commit_and_lower(self, inst, original_block, old_bb_map, bb_to_exit_bb)


tile.TileContext._commit_and_lower = _split_commit_and_lower


def build_nc() -> bass.Bass:
    nc = bass.Bass()
    yt_d = nc.declare_dram_parameter("yt", [N, BC], F32, isOutput=False)
    bt_d = nc.declare_dram_parameter("bt", [M, BC], F32, isOutput=False)
    at_d = nc.declare_dram_parameter("at", [N, M], F32, isOutput=False)
    w_d = nc.declare_dram_parameter("w", [M, N], F32, isOutput=False)
    out_d = nc.declare_dram_parameter("out", [N, BC], F32, isOutput=True)

    # dim-chunked 3D views: partition = row-within-chunk, then (chunk, batch)
    yt_v = yt_d.rearrange("(k p) b -> p k b", p=128)
    at_v = at_d.rearrange("(k p) m -> p k m", p=128)
    out_v = out_d.rearrange("(k p) b -> p k b", p=128)

    with tile.TileContext(nc) as tc:
        with (
            tc.tile_pool(name="const", bufs=1) as constp,
            tc.tile_pool(name="yts", bufs=NJ) as ytp,
            tc.tile_pool(name="tts", bufs=2) as ttp,
            tc.tile_pool(name="outs", bufs=4) as outp,
            tc.tile_pool(name="ps1", bufs=2, space="PSUM") as ps1,
            tc.tile_pool(name="ps2", bufs=3, space="PSUM") as ps2,
        ):
            at_s = constp.tile([128, KC, M], F32)  # A^T chunks: p=dim, free=m
            nc.sync.dma_start(_s1(at_s[:]), _s1(at_v[:]))
            w_s = constp.tile([128, N], F32)  # partition = m, free = dim
            nc.sync.dma_start(_s2(w_s[:]), _s2(w_d[:]))
            bt_s = constp.tile([128, BC], F32)  # partition = m, free = batch
            nc.sync.dma_start(bt_s[:], bt_d[:])

            # All input loads issue up front so the load stream is contiguous
            # on the DMA engines; compute for tile j starts as its load lands.
            ytjs = []
            for j in range(NJ):
                ytj = ytp.tile([128, KC, F], F32)
                nc.sync.dma_start(
                    _s1(ytj[:]), _s1(yt_v[:, :, j * F:(j + 1) * F])
                )
                ytjs.append(ytj)

            # j-major software pipeline: each batch tile of 512 flows
            # mm1(accum 8) -> sub -> 4x(2xmm2 -> sub) -> store-halves
            # independently, so input DMA, PE, DVE, and output DMA overlap.
            for j in range(NJ):
                ytj = ytjs[j]
                pt = ps1.tile([128, F], F32)
                for k in range(KC):
                    nc.tensor.matmul(
                        pt[:],
                        _s1(at_s[:, k, :]),
                        _s1(ytj[:, k, :]),
                        start=(k == 0),
                        stop=(k == KC - 1),
                    )
                tt = ttp.tile([128, F], F32)
                nc.vector.tensor_sub(
                    _s2(tt[:]), pt[:], bt_s[:, j * F:(j + 1) * F]
                )

                # stage 2 in pairs of d-chunks: 2-bank PSUM tiles halve the
                # DVE op count, and half-size output DMAs stream out earlier.
                for h in range(KC // 4):  # two halves of 4 d-chunks each
                    oh = outp.tile([128, KC // 2, F], F32)
                    for g in range(2):  # two d-pairs per half
                        p2 = ps2.tile([128, 2, F], F32)
                        for e in range(2):
                            d = h * 4 + g * 2 + e
                            nc.tensor.matmul(
                                p2[:, e, :],
                                _s2(w_s[:, d * 128:(d + 1) * 128]),
                                _s2(tt[:]),
                                start=True,
                                stop=True,
                            )
                        d0 = h * 4 + g * 2
                        nc.vector.tensor_sub(
                            oh[:, g * 2:(g + 1) * 2, :],
                            ytj[:, d0:d0 + 2, :],
                            p2[:],
                        )
                    # stores ride the scalar engine's HWDGE ring so they don't
                    # FIFO-queue behind the remaining input loads on sync
                    nc.scalar.dma_start(
                        out_v[:, h * 4:(h + 1) * 4, j * F:(j + 1) * F], oh[:]
                    )
    return nc


_NC_CACHE = None
_RUNNER = None


def _get_nc():
    global _NC_CACHE
    if _NC_CACHE is None:
        _NC_CACHE = build_nc()
    return _NC_CACHE


def _build_runner():
    """Persistent jitted shard_map callable over 8 cores (mirrors
    bass2jax.run_bass_via_pjrt's multi-core path, but cached so repeated
    kernel() calls skip retracing/XLA recompile)."""
    import jax
    from jax.sharding import Mesh, PartitionSpec
    from jax.experimental.shard_map import shard_map
    from concourse import bass2jax as b2j

    nc = _get_nc()
    b2j.install_neuronx_cc_hook()
    assert nc.dbg_addr is None
    partition_name = nc.partition_id_tensor.name if nc.partition_id_tensor else None

    in_names, out_names, out_avals, zero_shapes = [], [], [], []
    for alloc in nc.m.functions[0].allocations:
        if not isinstance(alloc, mybir.MemoryLocationSet):
            continue
        name = alloc.memorylocations[0].name
        if alloc.kind == "ExternalInput":
            if name != partition_name:
                in_names.append(name)
        elif alloc.kind == "ExternalOutput":
            out_names.append(name)
            shape = tuple(alloc.tensor_shape)
            dtype = mybir.dt.np(alloc.dtype)
            out_avals.append(jax.core.ShapedArray(shape, dtype))
            zero_shapes.append((shape, dtype))
    n_params = len(in_names)
    n_outs = len(out_names)
    all_in_names = tuple(in_names) + tuple(out_names)
    if partition_name is not None:
        all_in_names = all_in_names + (partition_name,)

    def _body(*args):
        operands = list(args)
        if partition_name is not None:
            operands.append(b2j.partition_id_tensor())
        outs = b2j._bass_exec_p.bind(
            *operands,
            out_avals=tuple(out_avals),
            in_names=all_in_names,
            out_names=tuple(out_names),
            lowering_input_output_aliases=(),
            sim_require_finite=True,
            sim_require_nnan=True,
            nc=nc,
        )
        return tuple(outs)

    devices = jax.devices()[:N_CORES]
    mesh = Mesh(np.asarray(devices), ("core",))
    in_specs = (PartitionSpec("core"),) * (n_params + n_outs)
    out_specs = (PartitionSpec("core"),) * n_outs
    donate = tuple(range(n_params, n_params + n_outs))
    sharded = jax.jit(
        shard_map(
            _body, mesh=mesh, in_specs=in_specs, out_specs=out_specs,
            check_rep=False,
        ),
        donate_argnums=donate,
        keep_unused=True,
    )

    from jax.sharding import NamedSharding

    zeros_fns = [
        jax.jit(
            lambda s=shape, d=dtype: jax.numpy.zeros(
                (N_CORES * s[0], *s[1:]), d
            ),
            out_shardings=NamedSharding(mesh, PartitionSpec("core")),
        )
        for shape, dtype in zero_shapes
    ]

    def run(named_inputs: dict):
        """named_inputs: name -> concatenated (N_CORES*dim0, ...) array."""
        ins = [named_inputs[n] for n in in_names]
        zeros = [f() for f in zeros_fns]
        outs = sharded(*ins, *zeros)
        return dict(zip(out_names, outs))

    run._parts = {
        "sharded": sharded,
        "in_names": in_names,
        "out_names": out_names,
        "mesh": mesh,
        "zeros_fns": zeros_fns,
    }
    return run


def _get_runner():
    global _RUNNER
    if _RUNNER is None:
        _RUNNER = _build_runner()
    return _RUNNER


def _prep_inputs(y, A, b):
    A64 = A.astype(np.float64)
    W = np.linalg.solve(A64 @ A64.T, A64).astype(np.float32)  # (M, N)
    AT = np.ascontiguousarray(A.T)  # (N, M)
    # concat-over-cores layouts expected by the shard_map runner
    yt_cat = np.ascontiguousarray(
        y.reshape(N_CORES, BC, N).transpose(0, 2, 1)
    ).reshape(N_CORES * N, BC)
    bt_cat = np.ascontiguousarray(
        b.reshape(N_CORES, BC, M).transpose(0, 2, 1)
    ).reshape(N_CORES * M, BC)
    at_cat = np.broadcast_to(AT, (N_CORES, N, M)).reshape(N_CORES * N, M)
    w_cat = np.broadcast_to(W, (N_CORES, M, N)).reshape(N_CORES * M, N)
    return {"yt": yt_cat, "bt": bt_cat, "at": at_cat, "w": w_cat}


def _unpack_output(out_cat: np.ndarray) -> np.ndarray:
    return np.ascontiguousarray(
        np.asarray(out_cat).reshape(N_CORES, N, BC).transpose(0, 2, 1)
    ).reshape(BATCH, N)


def kernel(y: np.ndarray, A: np.ndarray, b: np.ndarray) -> np.ndarray:
    y = np.ascontiguousarray(np.asarray(y, dtype=np.float32))
    A = np.ascontiguousarray(np.asarray(A, dtype=np.float32))
    b = np.ascontiguousarray(np.asarray(b, dtype=np.float32))
    assert y.shape == (BATCH, N) and A.shape == (M, N) and b.shape == (BATCH, M)

    named = _prep_inputs(y, A, b)
    try:
        run = _get_runner()
        out = run(named)["out"]
        return _unpack_output(out)
    except Exception:
        # Fallback: slower but uses only the public SPMD entry point.
        in_maps = [
            {
                k: np.ascontiguousarray(
                    v.reshape(N_CORES, v.shape[0] // N_CORES, *v.shape[1:])[i]
                )
                for k, v in named.items()
            }
            for i in range(N_CORES)
        ]
        res = run_bass_kernel_spmd(_get_nc(), in_maps, list(range(N_CORES)))
        x = np.empty((BATCH, N), dtype=np.float32)
        for i in range(N_CORES):
            x[i * BC:(i + 1) * BC, :] = res.results[i]["out"].T
        return x



# revision 13
# speedup vs baseline: 124.5925x; 124.5925x over previous
"""Constraint-projection layer on 8 Trainium2 NeuronCores.

Reference computes, per batch row y_i:  x_i = argmin ||x - y_i|| s.t. A x = b_i
via a dense KKT solve. Closed form (Schur complement of the KKT system):

    x = y - A^T (A A^T)^{-1} (A y - b)

Host precomputes W = (A A^T)^{-1} A  (128 x 1024, float64 solve). All device
I/O is fp16: y/b/out plus the small constants — this halves HBM traffic (the
kernel is DMA-bound) and runs the PE at 1 cycle/row instead of fp32's 4.
Accumulation stays fp32 in PSUM, so the only precision loss is the fp16
rounding of the operands (~1e-3 max rel err vs the 2e-2 gate).

Each core gets a 2048-row batch shard in TRANSPOSED layout (dim-major):

    stage 1:  T^T = A @ Y^T - B^T          (128 m  x 2048 batch)
    stage 2:  X^T = Y^T - W^T @ T^T        (1024 d x 2048 batch)

Stage 2 is split across engines per 512-batch tile so no one engine gates the
DMA stream: d-chunks 0-3 do PSUM = W^T T then DVE computes y - PSUM (fp16
out); d-chunks 4-7 accumulate PSUM = (-W)^T T + I^T Y on the PE and the
Activation engine just copies PSUM -> fp16 SBUF. The -W half is baked into
the uploaded W constant.

Data-parallel: no cross-core communication.
"""

import os

import numpy as np
import bass_rust as _br
import concourse.bass as bass
import concourse.mybir as mybir
from concourse import tile
from concourse.bass_utils import run_bass_kernel_spmd

F32 = mybir.dt.float32

# I/O dtype: fp16 default; bf16 fallback switch kept for HW-compile issues.
IO_MODE = os.environ.get("KERNEL_IO_DTYPE", "f16")
if IO_MODE == "bf16":
    F16 = mybir.dt.bfloat16
else:
    F16 = mybir.dt.float16


def _np_f16():
    if IO_MODE == "bf16":
        import ml_dtypes

        return np.dtype(ml_dtypes.bfloat16)
    return np.dtype(np.float16)

N_CORES = 8
BATCH = 16384
N = 1024           # input dim
M = 128            # constraint dim
BC = BATCH // N_CORES  # 2048 batch rows per core
KC = N // 128      # 8 contraction chunks
F = 512            # free-dim tile (one PSUM bank of f32)
NJ = BC // F       # 4 batch tiles per core
ND = 4             # d-chunks on the DVE-subtract path (rest use PE+Act copy)
N_WARM = 20        # dummy matmuls that pre-ramp the PE pstate clock


def _split_drain_and_barrier(self, tick_clock, wait_clock):
    # Walrus in this toolchain rejects >2 sync waits on the Tile tail Drain
    # (CTRL_NO_STRUCT). Emit one-wait-per-nop instructions ahead of the
    # drain instead; sequentially identical on the sync sequencer.
    gc = tick_clock.global_clock
    vals = eval(repr(gc).replace("VectorClock", "").strip("()"))
    for i, v in enumerate(vals):
        if v:
            single = [0] * len(vals)
            single[i] = v
            nop = self.nc.sync.nop(nofuse=True)
            wait_clock.add_sem_waits(
                nop.ins, _br.ScopedClock({None: _br.VectorClock(single)})
            )
    self.nc.sync.drain()
    self.nc.all_engine_barrier()
    assert self.sems is not None
    popped = self.nc._tile_sem_poison_stack.pop()
    assert popped is self._sem_poison
    self.nc.clear_and_free_semaphores(list(self.sems.allocated().values()))
    self.nc.all_engine_barrier()


tile.TileContext._drain_and_barrier = _split_drain_and_barrier

_orig_commit_and_lower = tile.TileContext._commit_and_lower

# Same walrus limitation for regular instructions: Matmult (S3_LW) takes no
# extra sync waits, most others take one. Spill excess waits onto dedicated
# same-engine nops committed immediately before the instruction.
_ZERO_WAIT_OPS = ("InstMatmult", "InstDrain")


def _split_commit_and_lower(self, inst, original_block, old_bb_map, bb_to_exit_bb):
    tn = type(inst).__name__
    if tn.startswith("Inst") and inst.engine is not None:
        si = inst.sync_info
        if si is not None:
            waits = list(si.on_wait)
            keep = 0 if tn in _ZERO_WAIT_OPS else 1
            if len(waits) > keep:
                spill, keep_waits = (
                    (waits, []) if keep == 0 else (waits[:-1], [waits[-1]])
                )
                for w_ in spill:
                    nop = mybir.InstNoOp(
                        name=self.nc.get_next_instruction_name(),
                        engine=inst.engine,
                        sync_info=mybir.SyncInfo(on_wait=[w_], on_update=[]),
                        bass_nofuse=True,
                    )
                    self._commit_instruction(nop)
                inst.sync_info = mybir.SyncInfo(
                    on_wait=keep_waits, on_update=list(si.on_update)
                )
    return _orig_commit_and_lower(self, inst, original_block, old_bb_map, bb_to_exit_bb)


tile.TileContext._commit_and_lower = _split_commit_and_lower


def build_nc() -> bass.Bass:
    nc = bass.Bass()
    yt_d = nc.declare_dram_parameter("yt", [N, BC], F16, isOutput=False)
    bt_d = nc.declare_dram_parameter("bt", [M, BC], F16, isOutput=False)
    at_d = nc.declare_dram_parameter("at", [128, KC, M], F16, isOutput=False)
    w_d = nc.declare_dram_parameter("w", [M, N], F16, isOutput=False)
    id_d = nc.declare_dram_parameter("idm", [128, 128], F16, isOutput=False)
    out_d = nc.declare_dram_parameter("out", [N, BC], F16, isOutput=True)

    # dim-chunked 3D views: partition = row-within-chunk, then (chunk, batch)
    yt_v = yt_d.rearrange("(k p) b -> p k b", p=128)
    out_v = out_d.rearrange("(k p) b -> p k b", p=128)

    with tile.TileContext(nc) as tc:
        with (
            tc.tile_pool(name="const", bufs=1) as constp,
            tc.tile_pool(name="yts", bufs=NJ) as ytp,
            tc.tile_pool(name="tts", bufs=3) as ttp,
            tc.tile_pool(name="outs", bufs=6) as outp,
            tc.tile_pool(name="ps1", bufs=2, space="PSUM") as ps1,
            tc.tile_pool(name="ps2", bufs=6, space="PSUM") as ps2,
        ):
            # PE pstate pre-ramp: the cost of a matmul depends on how long the
            # PE has been continuously busy (LOW->MID->FULL over ~3us). Dummy
            # zero matmuls starting right after the preamble put the engine at
            # FULL speed by the time the first real operand tile lands.
            wz = constp.tile([128, 384], F16)
            nc.gpsimd.memset(wz[:], 0.0)
            warm = ps1.tile([128, F], F32, name="pt")
            for _ in range(N_WARM):
                nc.tensor.matmul(
                    warm[:, 0:256], wz[:, 0:128], wz[:, 128:384],
                    start=True, stop=True,
                )

            # Load order: stage-1 operands first (at, then y j0), so the first
            # real matmul starts ~3us earlier than a consts-first order.
            at_s = constp.tile([128, KC, M], F16)  # A^T chunks: p=dim, free=m
            nc.sync.dma_start(at_s[:], at_d[:])
            ytjs = []
            for j in range(NJ):
                ytj = ytp.tile([128, KC, F], F16, name=f"ytj{j}")
                ytjs.append(ytj)
            # y tile loads are split in k-halves so stage-1 k=0..3 matmuls of
            # a batch tile can begin after half its load has landed.
            def load_y(j, h):
                nc.sync.dma_start(
                    ytjs[j][:, h * 4:(h + 1) * 4, :],
                    yt_v[:, h * 4:(h + 1) * 4, j * F:(j + 1) * F],
                )
            load_y(0, 0)
            load_y(0, 1)
            bt_s = constp.tile([128, BC], F16)  # partition = m, free = batch
            nc.sync.dma_start(bt_s[:], bt_d[:])
            w_s = constp.tile([128, N], F16)  # partition = m, free = dim
            nc.sync.dma_start(w_s[:], w_d[:])
            id_s = constp.tile([128, 128], F16)
            nc.sync.dma_start(id_s[:], id_d[:])
            for j in range(1, NJ):
                load_y(j, 0)
                load_y(j, 1)

            # Stage bodies. s1(j): 8 accumulating matmuls + the DVE
            # subtract/downcast that produces T in fp16. s2(j): the Act half
            # first (PE accumulates y - W^T T, Act only copies out of PSUM)
            # so the Activation engine starts early, then the DVE-subtract
            # half; each d-chunk pair streams out in its own 728ns store.
            tts = {}

            def s1(j):
                ytj = ytjs[j]
                pt = ps1.tile([128, F], F32, name="pt")
                for k in range(KC):
                    nc.tensor.matmul(
                        pt[:],
                        at_s[:, k, :],
                        ytj[:, k, :],
                        start=(k == 0),
                        stop=(k == KC - 1),
                    )
                tt = ttp.tile([128, F], F16, name="tt")
                nc.vector.tensor_sub(
                    tt[:], pt[:], bt_s[:, j * F:(j + 1) * F]
                )
                tts[j] = tt

            def s2(j):
                # Per-chunk stage 2: one PSUM bank per d-chunk (6-deep
                # rotation), consumer alternates DVE (even d: y - PSUM) and
                # Act (odd d: PE accumulates -W^T T + I^T Y, Act copies).
                # Finer grains keep every engine's idle gaps at sem latency.
                ytj = ytjs[j]
                tt = tts[j]
                oh = outp.tile([128, KC, F], F16, name="oh")
                for d in range(KC):
                    p2 = ps2.tile([128, F], F32, name="p2")
                    act = d % 2 == 1
                    nc.tensor.matmul(
                        p2[:],
                        w_s[:, d * 128:(d + 1) * 128],
                        tt[:],
                        start=True,
                        stop=not act,
                    )
                    if act:
                        nc.tensor.matmul(
                            p2[:],
                            id_s[:],
                            ytj[:, d, :],
                            start=False,
                            stop=True,
                        )
                        nc.scalar.copy(oh[:, d, :], p2[:])
                    else:
                        nc.vector.tensor_sub(
                            oh[:, d, :], ytj[:, d, :], p2[:]
                        )
                    if act:
                        nc.sync.dma_start(
                            out_v[:, d - 1:d + 1, j * F:(j + 1) * F],
                            oh[:, d - 1:d + 1, :],
                        )

            # One-j-lookahead software pipeline: the PE runs s1 of a later
            # batch tile while the DVE turns the previous tile's stage-1 PSUM
            # into fp16 T, so the PE never stalls waiting for T.
            s1(0)
            s2(0)
            s1(1)
            for j in range(2, NJ):
                s1(j)
                s2(j - 1)
            s2(NJ - 1)
    return nc


_NC_CACHE = None
_RUNNER = None


def _get_nc():
    global _NC_CACHE
    if _NC_CACHE is None:
        _NC_CACHE = build_nc()
    return _NC_CACHE


def _build_runner():
    """Persistent jitted shard_map callable over 8 cores (mirrors
    bass2jax.run_bass_via_pjrt's multi-core path, but cached so repeated
    kernel() calls skip retracing/XLA recompile)."""
    import jax
    from jax.sharding import Mesh, PartitionSpec
    from jax.experimental.shard_map import shard_map
    from concourse import bass2jax as b2j

    nc = _get_nc()
    b2j.install_neuronx_cc_hook()
    assert nc.dbg_addr is None
    partition_name = nc.partition_id_tensor.name if nc.partition_id_tensor else None

    in_names, out_names, out_avals, zero_shapes = [], [], [], []
    for alloc in nc.m.functions[0].allocations:
        if not isinstance(alloc, mybir.MemoryLocationSet):
            continue
        name = alloc.memorylocations[0].name
        if alloc.kind == "ExternalInput":
            if name != partition_name:
                in_names.append(name)
        elif alloc.kind == "ExternalOutput":
            out_names.append(name)
            shape = tuple(alloc.tensor_shape)
            dtype = mybir.dt.np(alloc.dtype)
            out_avals.append(jax.core.ShapedArray(shape, dtype))
            zero_shapes.append((shape, dtype))
    n_params = len(in_names)
    n_outs = len(out_names)
    all_in_names = tuple(in_names) + tuple(out_names)
    if partition_name is not None:
        all_in_names = all_in_names + (partition_name,)

    def _body(*args):
        operands = list(args)
        if partition_name is not None:
            operands.append(b2j.partition_id_tensor())
        outs = b2j._bass_exec_p.bind(
            *operands,
            out_avals=tuple(out_avals),
            in_names=all_in_names,
            out_names=tuple(out_names),
            lowering_input_output_aliases=(),
            sim_require_finite=True,
            sim_require_nnan=True,
            nc=nc,
        )
        return tuple(outs)

    devices = jax.devices()[:N_CORES]
    mesh = Mesh(np.asarray(devices), ("core",))
    in_specs = (PartitionSpec("core"),) * (n_params + n_outs)
    out_specs = (PartitionSpec("core"),) * n_outs
    donate = tuple(range(n_params, n_params + n_outs))
    sharded = jax.jit(
        shard_map(
            _body, mesh=mesh, in_specs=in_specs, out_specs=out_specs,
            check_rep=False,
        ),
        donate_argnums=donate,
        keep_unused=True,
    )

    from jax.sharding import NamedSharding

    zeros_fns = [
        jax.jit(
            lambda s=shape, d=dtype: jax.numpy.zeros(
                (N_CORES * s[0], *s[1:]), d
            ),
            out_shardings=NamedSharding(mesh, PartitionSpec("core")),
        )
        for shape, dtype in zero_shapes
    ]

    def run(named_inputs: dict):
        """named_inputs: name -> concatenated (N_CORES*dim0, ...) array."""
        ins = [named_inputs[n] for n in in_names]
        zeros = [f() for f in zeros_fns]
        outs = sharded(*ins, *zeros)
        return dict(zip(out_names, outs))

    run._parts = {
        "sharded": sharded,
        "in_names": in_names,
        "out_names": out_names,
        "mesh": mesh,
        "zeros_fns": zeros_fns,
    }
    return run


def _get_runner():
    global _RUNNER
    if _RUNNER is None:
        _RUNNER = _build_runner()
    return _RUNNER


def _prep_inputs(y, A, b):
    f16 = _np_f16()
    A64 = A.astype(np.float64)
    W = np.linalg.solve(A64 @ A64.T, A64)  # (M, N)
    # chunks on the PE-accumulate path (odd d) are negated on upload
    sign = np.repeat(np.where(np.arange(KC) % 2 == 1, -1.0, 1.0), 128)
    W_mixed = (W * sign[None, :]).astype(f16)
    # at[p, k, m] = A[m, k*128+p]: stage-1 stationary chunks, contiguous rows
    at_p = np.ascontiguousarray(
        A.reshape(M, KC, 128).transpose(2, 1, 0)
    ).astype(f16)
    idm = np.eye(128).astype(f16)
    y16 = y.astype(f16)
    b16 = b.astype(f16)
    # concat-over-cores layouts expected by the shard_map runner
    yt_cat = np.ascontiguousarray(
        y16.reshape(N_CORES, BC, N).transpose(0, 2, 1)
    ).reshape(N_CORES * N, BC)
    bt_cat = np.ascontiguousarray(
        b16.reshape(N_CORES, BC, M).transpose(0, 2, 1)
    ).reshape(N_CORES * M, BC)
    at_cat = np.broadcast_to(at_p, (N_CORES, 128, KC, M)).reshape(
        N_CORES * 128, KC, M
    )
    w_cat = np.broadcast_to(W_mixed, (N_CORES, M, N)).reshape(N_CORES * M, N)
    id_cat = np.broadcast_to(idm, (N_CORES, 128, 128)).reshape(
        N_CORES * 128, 128
    )
    return {
        "yt": yt_cat, "bt": bt_cat, "at": at_cat, "w": w_cat, "idm": id_cat
    }


def _unpack_output(out_cat: np.ndarray) -> np.ndarray:
    return (
        np.asarray(out_cat)
        .reshape(N_CORES, N, BC)
        .transpose(0, 2, 1)
        .astype(np.float32)
        .reshape(BATCH, N)
    )


def kernel(y: np.ndarray, A: np.ndarray, b: np.ndarray) -> np.ndarray:
    y = np.ascontiguousarray(np.asarray(y, dtype=np.float32))
    A = np.ascontiguousarray(np.asarray(A, dtype=np.float32))
    b = np.ascontiguousarray(np.asarray(b, dtype=np.float32))
    assert y.shape == (BATCH, N) and A.shape == (M, N) and b.shape == (BATCH, M)

    named = _prep_inputs(y, A, b)
    try:
        run = _get_runner()
        out = run(named)["out"]
        return _unpack_output(out)
    except Exception:
        # Fallback: slower but uses only the public SPMD entry point.
        in_maps = [
            {
                k: np.ascontiguousarray(
                    v.reshape(N_CORES, v.shape[0] // N_CORES, *v.shape[1:])[i]
                )
                for k, v in named.items()
            }
            for i in range(N_CORES)
        ]
        res = run_bass_kernel_spmd(_get_nc(), in_maps, list(range(N_CORES)))
        x = np.empty((BATCH, N), dtype=np.float32)
        for i in range(N_CORES):
            x[i * BC:(i + 1) * BC, :] = (
                np.asarray(res.results[i]["out"]).T.astype(np.float32)
            )
        return x


# revision 19
# speedup vs baseline: 127.7606x; 1.0254x over previous
"""Constraint-projection layer on 8 Trainium2 NeuronCores.

Reference computes, per batch row y_i:  x_i = argmin ||x - y_i|| s.t. A x = b_i
via a dense KKT solve. Closed form (Schur complement of the KKT system):

    x = y - A^T (A A^T)^{-1} (A y - b)

Host precomputes W = (A A^T)^{-1} A  (128 x 1024, float64 solve). All device
I/O is fp16: y/b/out plus the small constants — this halves HBM traffic (the
kernel is DMA-bound) and runs the PE at 1 cycle/row instead of fp32's 4.
Accumulation stays fp32 in PSUM, so the only precision loss is the fp16
rounding of the operands (~1e-3 max rel err vs the 2e-2 gate).

Each core gets a 2048-row batch shard in TRANSPOSED layout (dim-major):

    stage 1:  T^T = A @ Y^T - B^T          (128 m  x 2048 batch)
    stage 2:  X^T = Y^T - W^T @ T^T        (1024 d x 2048 batch)

Stage 2 is split across engines per 512-batch tile so no one engine gates the
DMA stream: d-chunks 0-3 do PSUM = W^T T then DVE computes y - PSUM (fp16
out); d-chunks 4-7 accumulate PSUM = (-W)^T T + I^T Y on the PE and the
Activation engine just copies PSUM -> fp16 SBUF. The -W half is baked into
the uploaded W constant.

Data-parallel: no cross-core communication.
"""

import os

import numpy as np
import bass_rust as _br
import concourse.bass as bass
import concourse.mybir as mybir
from concourse import tile
from concourse.bass_utils import run_bass_kernel_spmd

F32 = mybir.dt.float32
F8 = mybir.dt.float8e4  # e4m3: carries b, whose error reaches x through
                        # A^T(AA^T)^{-1} with gain ~1/20 — ~4e-4 rel worst

# I/O dtype: fp16 default; bf16 fallback switch kept for HW-compile issues.
IO_MODE = os.environ.get("KERNEL_IO_DTYPE", "f16")
if IO_MODE == "bf16":
    F16 = mybir.dt.bfloat16
else:
    F16 = mybir.dt.float16


def _np_f16():
    if IO_MODE == "bf16":
        import ml_dtypes

        return np.dtype(ml_dtypes.bfloat16)
    return np.dtype(np.float16)

N_CORES = 8
BATCH = 16384
N = 1024           # input dim
M = 128            # constraint dim
BC = BATCH // N_CORES  # 2048 batch rows per core
KC = N // 128      # 8 contraction chunks
F = 512            # free-dim tile (one PSUM bank of f32)
NJ = BC // F       # 4 batch tiles per core
ND = 4             # d-chunks on the DVE-subtract path (rest use PE+Act copy)
N_WARM = 20        # dummy matmuls that pre-ramp the PE pstate clock


def _split_drain_and_barrier(self, tick_clock, wait_clock):
    # Walrus in this toolchain rejects >2 sync waits on the Tile tail Drain
    # (CTRL_NO_STRUCT). Emit one-wait-per-nop instructions ahead of the
    # drain instead; sequentially identical on the sync sequencer.
    gc = tick_clock.global_clock
    vals = eval(repr(gc).replace("VectorClock", "").strip("()"))
    for i, v in enumerate(vals):
        if v:
            single = [0] * len(vals)
            single[i] = v
            nop = self.nc.sync.nop(nofuse=True)
            wait_clock.add_sem_waits(
                nop.ins, _br.ScopedClock({None: _br.VectorClock(single)})
            )
    self.nc.sync.drain()
    self.nc.all_engine_barrier()
    assert self.sems is not None
    popped = self.nc._tile_sem_poison_stack.pop()
    assert popped is self._sem_poison
    self.nc.clear_and_free_semaphores(list(self.sems.allocated().values()))
    self.nc.all_engine_barrier()


tile.TileContext._drain_and_barrier = _split_drain_and_barrier

_orig_commit_and_lower = tile.TileContext._commit_and_lower

# Same walrus limitation for regular instructions: Matmult (S3_LW) takes no
# extra sync waits, most others take one. Spill excess waits onto dedicated
# same-engine nops committed immediately before the instruction.
_ZERO_WAIT_OPS = ("InstMatmult", "InstDrain")


def _split_commit_and_lower(self, inst, original_block, old_bb_map, bb_to_exit_bb):
    tn = type(inst).__name__
    if tn.startswith("Inst") and inst.engine is not None:
        si = inst.sync_info
        if si is not None:
            waits = list(si.on_wait)
            keep = 0 if tn in _ZERO_WAIT_OPS else 1
            if len(waits) > keep:
                spill, keep_waits = (
                    (waits, []) if keep == 0 else (waits[:-1], [waits[-1]])
                )
                for w_ in spill:
                    nop = mybir.InstNoOp(
                        name=self.nc.get_next_instruction_name(),
                        engine=inst.engine,
                        sync_info=mybir.SyncInfo(on_wait=[w_], on_update=[]),
                        bass_nofuse=True,
                    )
                    self._commit_instruction(nop)
                inst.sync_info = mybir.SyncInfo(
                    on_wait=keep_waits, on_update=list(si.on_update)
                )
    return _orig_commit_and_lower(self, inst, original_block, old_bb_map, bb_to_exit_bb)


tile.TileContext._commit_and_lower = _split_commit_and_lower


def build_nc() -> bass.Bass:
    nc = bass.Bass()
    yt_d = nc.declare_dram_parameter("yt", [N, BC], F16, isOutput=False)
    bt_d = nc.declare_dram_parameter("bt", [M, BC], F8, isOutput=False)
    at_d = nc.declare_dram_parameter("at", [128, KC, M], F16, isOutput=False)
    w_d = nc.declare_dram_parameter("w", [M, N], F16, isOutput=False)
    out_d = nc.declare_dram_parameter("out", [N, BC], F16, isOutput=True)

    # dim-chunked 3D views: partition = row-within-chunk, then (chunk, batch)
    yt_v = yt_d.rearrange("(k p) b -> p k b", p=128)
    out_v = out_d.rearrange("(k p) b -> p k b", p=128)

    with tile.TileContext(nc) as tc:
        with (
            tc.tile_pool(name="const", bufs=1) as constp,
            tc.tile_pool(name="yts", bufs=NJ) as ytp,
            tc.tile_pool(name="tts", bufs=3) as ttp,
            tc.tile_pool(name="outs", bufs=6) as outp,
            tc.tile_pool(name="ps1", bufs=2, space="PSUM") as ps1,
            tc.tile_pool(name="ps2", bufs=6, space="PSUM") as ps2,
        ):
            # PE pstate pre-ramp: the cost of a matmul depends on how long the
            # PE has been continuously busy (LOW->MID->FULL over ~3us). Dummy
            # zero matmuls starting right after the preamble put the engine at
            # FULL speed by the time the first real operand tile lands.
            wz = constp.tile([128, 384], F16)
            nc.gpsimd.memset(wz[:], 0.0)
            warm = ps1.tile([128, F], F32, name="pt")
            for _ in range(N_WARM):
                nc.tensor.matmul(
                    warm[:, 0:256], wz[:, 0:128], wz[:, 128:384],
                    start=True, stop=True,
                )

            # Load order: stage-1 operands first (at, then y j0), so the first
            # real matmul starts ~3us earlier than a consts-first order.
            at_s = constp.tile([128, KC, M], F16)  # A^T chunks: p=dim, free=m
            nc.sync.dma_start(at_s[:], at_d[:])
            ytjs = []
            for j in range(NJ):
                ytj = ytp.tile([128, KC, F], F16, name=f"ytj{j}")
                ytjs.append(ytj)
            # y tile loads are split in k-halves so stage-1 k=0..3 matmuls of
            # a batch tile can begin after half its load has landed.
            def load_y(j, h):
                nc.sync.dma_start(
                    ytjs[j][:, h * 4:(h + 1) * 4, :],
                    yt_v[:, h * 4:(h + 1) * 4, j * F:(j + 1) * F],
                )
            load_y(0, 0)
            load_y(0, 1)
            bt_s = constp.tile([128, BC], F8)  # partition = m, free = batch
            nc.sync.dma_start(bt_s[:], bt_d[:])
            w_s = constp.tile([128, N], F16)  # partition = m, free = dim
            nc.sync.dma_start(w_s[:], w_d[:])
            # 128x128 fp16 identity built in place: ones, then keep only the
            # diagonal (iota value i - p == 0), zero elsewhere.
            id_s = constp.tile([128, 128], F16)
            nc.gpsimd.memset(id_s[:], 1.0)
            nc.gpsimd.affine_select(
                id_s[:], id_s[:], [[1, 128]],
                mybir.AluOpType.is_equal, 0.0,
                base=0, channel_multiplier=-1,
            )
            for j in range(1, NJ):
                load_y(j, 0)
                load_y(j, 1)

            # Stage bodies. s1(j): 8 accumulating matmuls + the DVE
            # subtract/downcast that produces T in fp16. s2(j): the Act half
            # first (PE accumulates y - W^T T, Act only copies out of PSUM)
            # so the Activation engine starts early, then the DVE-subtract
            # half; each d-chunk pair streams out in its own 728ns store.
            tts = {}

            def s1(j):
                ytj = ytjs[j]
                pt = ps1.tile([128, F], F32, name="pt")
                for k in range(KC):
                    nc.tensor.matmul(
                        pt[:],
                        at_s[:, k, :],
                        ytj[:, k, :],
                        start=(k == 0),
                        stop=(k == KC - 1),
                    )
                tt = ttp.tile([128, F], F16, name="tt")
                nc.vector.tensor_sub(
                    tt[:], pt[:], bt_s[:, j * F:(j + 1) * F]
                )
                tts[j] = tt

            def s2(j):
                # Per-chunk stage 2: one PSUM bank per d-chunk (6-deep
                # rotation), consumer alternates DVE (even d: y - PSUM) and
                # Act (odd d: PE accumulates -W^T T + I^T Y, Act copies).
                # Finer grains keep every engine's idle gaps at sem latency.
                ytj = ytjs[j]
                tt = tts[j]
                oh = outp.tile([128, KC, F], F16, name="oh")
                for d in range(KC):
                    p2 = ps2.tile([128, F], F32, name="p2")
                    act = d % 2 == 1
                    nc.tensor.matmul(
                        p2[:],
                        w_s[:, d * 128:(d + 1) * 128],
                        tt[:],
                        start=True,
                        stop=not act,
                    )
                    if act:
                        nc.tensor.matmul(
                            p2[:],
                            id_s[:],
                            ytj[:, d, :],
                            start=False,
                            stop=True,
                        )
                        nc.scalar.copy(oh[:, d, :], p2[:])
                    else:
                        nc.vector.tensor_sub(
                            oh[:, d, :], ytj[:, d, :], p2[:]
                        )
                    if act:
                        nc.sync.dma_start(
                            out_v[:, d - 1:d + 1, j * F:(j + 1) * F],
                            oh[:, d - 1:d + 1, :],
                        )

            # One-j-lookahead software pipeline: the PE runs s1 of a later
            # batch tile while the DVE turns the previous tile's stage-1 PSUM
            # into fp16 T, so the PE never stalls waiting for T.
            s1(0)
            s2(0)
            s1(1)
            for j in range(2, NJ):
                s1(j)
                s2(j - 1)
            s2(NJ - 1)
    return nc


_NC_CACHE = None
_RUNNER = None


def _get_nc():
    global _NC_CACHE
    if _NC_CACHE is None:
        _NC_CACHE = build_nc()
    return _NC_CACHE


def _build_runner():
    """Persistent jitted shard_map callable over 8 cores (mirrors
    bass2jax.run_bass_via_pjrt's multi-core path, but cached so repeated
    kernel() calls skip retracing/XLA recompile)."""
    import jax
    from jax.sharding import Mesh, PartitionSpec
    from jax.experimental.shard_map import shard_map
    from concourse import bass2jax as b2j

    nc = _get_nc()
    b2j.install_neuronx_cc_hook()
    assert nc.dbg_addr is None
    partition_name = nc.partition_id_tensor.name if nc.partition_id_tensor else None

    in_names, out_names, out_avals, zero_shapes = [], [], [], []
    for alloc in nc.m.functions[0].allocations:
        if not isinstance(alloc, mybir.MemoryLocationSet):
            continue
        name = alloc.memorylocations[0].name
        if alloc.kind == "ExternalInput":
            if name != partition_name:
                in_names.append(name)
        elif alloc.kind == "ExternalOutput":
            out_names.append(name)
            shape = tuple(alloc.tensor_shape)
            dtype = mybir.dt.np(alloc.dtype)
            out_avals.append(jax.core.ShapedArray(shape, dtype))
            zero_shapes.append((shape, dtype))
    n_params = len(in_names)
    n_outs = len(out_names)
    all_in_names = tuple(in_names) + tuple(out_names)
    if partition_name is not None:
        all_in_names = all_in_names + (partition_name,)

    def _body(*args):
        operands = list(args)
        if partition_name is not None:
            operands.append(b2j.partition_id_tensor())
        outs = b2j._bass_exec_p.bind(
            *operands,
            out_avals=tuple(out_avals),
            in_names=all_in_names,
            out_names=tuple(out_names),
            lowering_input_output_aliases=(),
            sim_require_finite=True,
            sim_require_nnan=True,
            nc=nc,
        )
        return tuple(outs)

    devices = jax.devices()[:N_CORES]
    mesh = Mesh(np.asarray(devices), ("core",))
    in_specs = (PartitionSpec("core"),) * (n_params + n_outs)
    out_specs = (PartitionSpec("core"),) * n_outs
    donate = tuple(range(n_params, n_params + n_outs))
    sharded = jax.jit(
        shard_map(
            _body, mesh=mesh, in_specs=in_specs, out_specs=out_specs,
            check_rep=False,
        ),
        donate_argnums=donate,
        keep_unused=True,
    )

    from jax.sharding import NamedSharding

    zeros_fns = [
        jax.jit(
            lambda s=shape, d=dtype: jax.numpy.zeros(
                (N_CORES * s[0], *s[1:]), d
            ),
            out_shardings=NamedSharding(mesh, PartitionSpec("core")),
        )
        for shape, dtype in zero_shapes
    ]

    def run(named_inputs: dict):
        """named_inputs: name -> concatenated (N_CORES*dim0, ...) array."""
        ins = [named_inputs[n] for n in in_names]
        zeros = [f() for f in zeros_fns]
        outs = sharded(*ins, *zeros)
        return dict(zip(out_names, outs))

    run._parts = {
        "sharded": sharded,
        "in_names": in_names,
        "out_names": out_names,
        "mesh": mesh,
        "zeros_fns": zeros_fns,
    }
    return run


def _get_runner():
    global _RUNNER
    if _RUNNER is None:
        _RUNNER = _build_runner()
    return _RUNNER


def _prep_inputs(y, A, b):
    f16 = _np_f16()
    A64 = A.astype(np.float64)
    W = np.linalg.solve(A64 @ A64.T, A64)  # (M, N)
    # chunks on the PE-accumulate path (odd d) are negated on upload
    sign = np.repeat(np.where(np.arange(KC) % 2 == 1, -1.0, 1.0), 128)
    W_mixed = (W * sign[None, :]).astype(f16)
    # at[p, k, m] = A[m, k*128+p]: stage-1 stationary chunks, contiguous rows
    at_p = np.ascontiguousarray(
        A.reshape(M, KC, 128).transpose(2, 1, 0)
    ).astype(f16)
    f8 = np.dtype(mybir.dt.np(F8))
    y16 = y.astype(f16)
    b16 = b.astype(f8)
    # concat-over-cores layouts expected by the shard_map runner
    yt_cat = np.ascontiguousarray(
        y16.reshape(N_CORES, BC, N).transpose(0, 2, 1)
    ).reshape(N_CORES * N, BC)
    bt_cat = np.ascontiguousarray(
        b16.reshape(N_CORES, BC, M).transpose(0, 2, 1)
    ).reshape(N_CORES * M, BC)
    at_cat = np.broadcast_to(at_p, (N_CORES, 128, KC, M)).reshape(
        N_CORES * 128, KC, M
    )
    w_cat = np.broadcast_to(W_mixed, (N_CORES, M, N)).reshape(N_CORES * M, N)
    return {"yt": yt_cat, "bt": bt_cat, "at": at_cat, "w": w_cat}


def _unpack_output(out_cat: np.ndarray) -> np.ndarray:
    return (
        np.asarray(out_cat)
        .reshape(N_CORES, N, BC)
        .transpose(0, 2, 1)
        .astype(np.float32)
        .reshape(BATCH, N)
    )


def kernel(y: np.ndarray, A: np.ndarray, b: np.ndarray) -> np.ndarray:
    y = np.ascontiguousarray(np.asarray(y, dtype=np.float32))
    A = np.ascontiguousarray(np.asarray(A, dtype=np.float32))
    b = np.ascontiguousarray(np.asarray(b, dtype=np.float32))
    assert y.shape == (BATCH, N) and A.shape == (M, N) and b.shape == (BATCH, M)

    named = _prep_inputs(y, A, b)
    try:
        run = _get_runner()
        out = run(named)["out"]
        return _unpack_output(out)
    except Exception:
        # Fallback: slower but uses only the public SPMD entry point.
        in_maps = [
            {
                k: np.ascontiguousarray(
                    v.reshape(N_CORES, v.shape[0] // N_CORES, *v.shape[1:])[i]
                )
                for k, v in named.items()
            }
            for i in range(N_CORES)
        ]
        res = run_bass_kernel_spmd(_get_nc(), in_maps, list(range(N_CORES)))
        x = np.empty((BATCH, N), dtype=np.float32)
        for i in range(N_CORES):
            x[i * BC:(i + 1) * BC, :] = (
                np.asarray(res.results[i]["out"]).T.astype(np.float32)
            )
        return x


# revision 22
# speedup vs baseline: 128.8967x; 1.0089x over previous
"""Constraint-projection layer on 8 Trainium2 NeuronCores.

Reference computes, per batch row y_i:  x_i = argmin ||x - y_i|| s.t. A x = b_i
via a dense KKT solve. Closed form (Schur complement of the KKT system):

    x = y - A^T (A A^T)^{-1} (A y - b)

Host precomputes W = (A A^T)^{-1} A  (128 x 1024, float64 solve). All device
I/O is fp16: y/b/out plus the small constants — this halves HBM traffic (the
kernel is DMA-bound) and runs the PE at 1 cycle/row instead of fp32's 4.
Accumulation stays fp32 in PSUM, so the only precision loss is the fp16
rounding of the operands (~1e-3 max rel err vs the 2e-2 gate).

Each core gets a 2048-row batch shard in TRANSPOSED layout (dim-major):

    stage 1:  T^T = A @ Y^T - B^T          (128 m  x 2048 batch)
    stage 2:  X^T = Y^T - W^T @ T^T        (1024 d x 2048 batch)

Stage 2 is split across engines per 512-batch tile so no one engine gates the
DMA stream: d-chunks 0-3 do PSUM = W^T T then DVE computes y - PSUM (fp16
out); d-chunks 4-7 accumulate PSUM = (-W)^T T + I^T Y on the PE and the
Activation engine just copies PSUM -> fp16 SBUF. The -W half is baked into
the uploaded W constant.

Data-parallel: no cross-core communication.
"""

import os

import numpy as np
import bass_rust as _br
import concourse.bass as bass
import concourse.mybir as mybir
from concourse import tile
from concourse.bass_utils import run_bass_kernel_spmd

F32 = mybir.dt.float32
F8 = mybir.dt.float8e4  # e4m3: carries b, whose error reaches x through
                        # A^T(AA^T)^{-1} with gain ~1/20 — ~4e-4 rel worst

# I/O dtype: fp16 default; bf16 fallback switch kept for HW-compile issues.
IO_MODE = os.environ.get("KERNEL_IO_DTYPE", "f16")
if IO_MODE == "bf16":
    F16 = mybir.dt.bfloat16
else:
    F16 = mybir.dt.float16


def _np_f16():
    if IO_MODE == "bf16":
        import ml_dtypes

        return np.dtype(ml_dtypes.bfloat16)
    return np.dtype(np.float16)

N_CORES = 8
BATCH = 16384
N = 1024           # input dim
M = 128            # constraint dim
BC = BATCH // N_CORES  # 2048 batch rows per core
KC = N // 128      # 8 contraction chunks
F = 512            # free-dim tile (one PSUM bank of f32)
NJ = BC // F       # 4 batch tiles per core
ND = 4             # d-chunks on the DVE-subtract path (rest use PE+Act copy)
N_WARM = 20        # dummy matmuls that pre-ramp the PE pstate clock


def _split_drain_and_barrier(self, tick_clock, wait_clock):
    # Walrus in this toolchain rejects >2 sync waits on the Tile tail Drain
    # (CTRL_NO_STRUCT). Emit one-wait-per-nop instructions ahead of the
    # drain instead; sequentially identical on the sync sequencer.
    gc = tick_clock.global_clock
    vals = eval(repr(gc).replace("VectorClock", "").strip("()"))
    for i, v in enumerate(vals):
        if v:
            single = [0] * len(vals)
            single[i] = v
            nop = self.nc.sync.nop(nofuse=True)
            wait_clock.add_sem_waits(
                nop.ins, _br.ScopedClock({None: _br.VectorClock(single)})
            )
    self.nc.sync.drain()
    self.nc.all_engine_barrier()
    assert self.sems is not None
    popped = self.nc._tile_sem_poison_stack.pop()
    assert popped is self._sem_poison
    # No trailing barrier after the sem clears: every engine is quiesced by
    # the barrier above, and the clearing engine halts after its own clears,
    # so re-execution still sees zeroed semaphores.
    self.nc.clear_and_free_semaphores(list(self.sems.allocated().values()))


tile.TileContext._drain_and_barrier = _split_drain_and_barrier

_orig_commit_and_lower = tile.TileContext._commit_and_lower

# Same walrus limitation for regular instructions: Matmult (S3_LW) takes no
# extra sync waits, most others take one. Spill excess waits onto dedicated
# same-engine nops committed immediately before the instruction.
_ZERO_WAIT_OPS = ("InstMatmult", "InstDrain")


def _split_commit_and_lower(self, inst, original_block, old_bb_map, bb_to_exit_bb):
    tn = type(inst).__name__
    if tn.startswith("Inst") and inst.engine is not None:
        si = inst.sync_info
        if si is not None:
            waits = list(si.on_wait)
            keep = 0 if tn in _ZERO_WAIT_OPS else 1
            if len(waits) > keep:
                spill, keep_waits = (
                    (waits, []) if keep == 0 else (waits[:-1], [waits[-1]])
                )
                for w_ in spill:
                    nop = mybir.InstNoOp(
                        name=self.nc.get_next_instruction_name(),
                        engine=inst.engine,
                        sync_info=mybir.SyncInfo(on_wait=[w_], on_update=[]),
                        bass_nofuse=True,
                    )
                    self._commit_instruction(nop)
                inst.sync_info = mybir.SyncInfo(
                    on_wait=keep_waits, on_update=list(si.on_update)
                )
    return _orig_commit_and_lower(self, inst, original_block, old_bb_map, bb_to_exit_bb)


tile.TileContext._commit_and_lower = _split_commit_and_lower


def build_nc() -> bass.Bass:
    nc = bass.Bass()
    yt_d = nc.declare_dram_parameter("yt", [N, BC], F16, isOutput=False)
    bt_d = nc.declare_dram_parameter("bt", [M, BC], F8, isOutput=False)
    at_d = nc.declare_dram_parameter("at", [128, KC, M], F16, isOutput=False)
    w_d = nc.declare_dram_parameter("w", [M, N], F16, isOutput=False)
    out_d = nc.declare_dram_parameter("out", [N, BC], F16, isOutput=True)

    # dim-chunked 3D views: partition = row-within-chunk, then (chunk, batch)
    yt_v = yt_d.rearrange("(k p) b -> p k b", p=128)
    out_v = out_d.rearrange("(k p) b -> p k b", p=128)

    with tile.TileContext(nc) as tc:
        with (
            tc.tile_pool(name="const", bufs=1) as constp,
            tc.tile_pool(name="yts", bufs=NJ) as ytp,
            tc.tile_pool(name="tts", bufs=3) as ttp,
            tc.tile_pool(name="outs", bufs=6) as outp,
            tc.tile_pool(name="ps1", bufs=2, space="PSUM") as ps1,
            tc.tile_pool(name="ps2", bufs=6, space="PSUM") as ps2,
        ):
            # PE pstate pre-ramp: the cost of a matmul depends on how long the
            # PE has been continuously busy (LOW->MID->FULL over ~3us). Dummy
            # zero matmuls starting right after the preamble put the engine at
            # FULL speed by the time the first real operand tile lands.
            wz = constp.tile([128, 384], F16)
            nc.gpsimd.memset(wz[:], 0.0)
            warm = ps1.tile([128, F], F32, name="pt")
            for _ in range(N_WARM):
                nc.tensor.matmul(
                    warm[:, 0:256], wz[:, 0:128], wz[:, 128:384],
                    start=True, stop=True,
                )

            # Load order: stage-1 operands first (at, then y j0), so the first
            # real matmul starts ~3us earlier than a consts-first order.
            at_s = constp.tile([128, KC, M], F16)  # A^T chunks: p=dim, free=m
            nc.sync.dma_start(at_s[:], at_d[:])
            ytjs = []
            for j in range(NJ):
                ytj = ytp.tile([128, KC, F], F16, name=f"ytj{j}")
                ytjs.append(ytj)
            # y tile loads are split in k-halves so stage-1 k=0..3 matmuls of
            # a batch tile can begin after half its load has landed.
            def load_y(j, h):
                nc.sync.dma_start(
                    ytjs[j][:, h * 4:(h + 1) * 4, :],
                    yt_v[:, h * 4:(h + 1) * 4, j * F:(j + 1) * F],
                )
            load_y(0, 0)
            load_y(0, 1)
            bt_s = constp.tile([128, BC], F8)  # partition = m, free = batch
            nc.sync.dma_start(bt_s[:], bt_d[:])
            w_s = constp.tile([128, N], F16)  # partition = m, free = dim
            nc.sync.dma_start(w_s[:], w_d[:])
            # 128x128 fp16 identity built in place: ones, then keep only the
            # diagonal (iota value i - p == 0), zero elsewhere.
            id_s = constp.tile([128, 128], F16)
            nc.gpsimd.memset(id_s[:], 1.0)
            nc.gpsimd.affine_select(
                id_s[:], id_s[:], [[1, 128]],
                mybir.AluOpType.is_equal, 0.0,
                base=0, channel_multiplier=-1,
            )
            for j in range(1, NJ):
                load_y(j, 0)
                load_y(j, 1)

            # Stage bodies. s1(j): 8 accumulating matmuls + the DVE
            # subtract/downcast that produces T in fp16. s2(j): the Act half
            # first (PE accumulates y - W^T T, Act only copies out of PSUM)
            # so the Activation engine starts early, then the DVE-subtract
            # half; each d-chunk pair streams out in its own 728ns store.
            tts = {}

            def s1(j):
                ytj = ytjs[j]
                pt = ps1.tile([128, F], F32, name="pt")
                for k in range(KC):
                    nc.tensor.matmul(
                        pt[:],
                        at_s[:, k, :],
                        ytj[:, k, :],
                        start=(k == 0),
                        stop=(k == KC - 1),
                    )
                tt = ttp.tile([128, F], F16, name="tt")
                nc.vector.tensor_sub(
                    tt[:], pt[:], bt_s[:, j * F:(j + 1) * F]
                )
                tts[j] = tt

            def s2(j):
                # Per-chunk stage 2: one PSUM bank per d-chunk (6-deep
                # rotation), consumer alternates DVE (even d: y - PSUM) and
                # Act (odd d: PE accumulates -W^T T + I^T Y, Act copies).
                # Finer grains keep every engine's idle gaps at sem latency.
                ytj = ytjs[j]
                tt = tts[j]
                oh = outp.tile([128, KC, F], F16, name="oh")
                for d in range(KC):
                    p2 = ps2.tile([128, F], F32, name="p2")
                    act = d % 2 == 1
                    nc.tensor.matmul(
                        p2[:],
                        w_s[:, d * 128:(d + 1) * 128],
                        tt[:],
                        start=True,
                        stop=not act,
                    )
                    if act:
                        nc.tensor.matmul(
                            p2[:],
                            id_s[:],
                            ytj[:, d, :],
                            start=False,
                            stop=True,
                        )
                        nc.scalar.copy(oh[:, d, :], p2[:])
                    else:
                        nc.vector.tensor_sub(
                            oh[:, d, :], ytj[:, d, :], p2[:]
                        )
                    if act:
                        nc.sync.dma_start(
                            out_v[:, d - 1:d + 1, j * F:(j + 1) * F],
                            oh[:, d - 1:d + 1, :],
                        )

            # One-j-lookahead software pipeline: the PE runs s1 of a later
            # batch tile while the DVE turns the previous tile's stage-1 PSUM
            # into fp16 T, so the PE never stalls waiting for T.
            s1(0)
            s2(0)
            s1(1)
            for j in range(2, NJ):
                s1(j)
                s2(j - 1)
            s2(NJ - 1)
    return nc


_NC_CACHE = None
_RUNNER = None


def _get_nc():
    global _NC_CACHE
    if _NC_CACHE is None:
        _NC_CACHE = build_nc()
    return _NC_CACHE


def _build_runner():
    """Persistent jitted shard_map callable over 8 cores (mirrors
    bass2jax.run_bass_via_pjrt's multi-core path, but cached so repeated
    kernel() calls skip retracing/XLA recompile)."""
    import jax
    from jax.sharding import Mesh, PartitionSpec
    from jax.experimental.shard_map import shard_map
    from concourse import bass2jax as b2j

    nc = _get_nc()
    b2j.install_neuronx_cc_hook()
    assert nc.dbg_addr is None
    partition_name = nc.partition_id_tensor.name if nc.partition_id_tensor else None

    in_names, out_names, out_avals, zero_shapes = [], [], [], []
    for alloc in nc.m.functions[0].allocations:
        if not isinstance(alloc, mybir.MemoryLocationSet):
            continue
        name = alloc.memorylocations[0].name
        if alloc.kind == "ExternalInput":
            if name != partition_name:
                in_names.append(name)
        elif alloc.kind == "ExternalOutput":
            out_names.append(name)
            shape = tuple(alloc.tensor_shape)
            dtype = mybir.dt.np(alloc.dtype)
            out_avals.append(jax.core.ShapedArray(shape, dtype))
            zero_shapes.append((shape, dtype))
    n_params = len(in_names)
    n_outs = len(out_names)
    all_in_names = tuple(in_names) + tuple(out_names)
    if partition_name is not None:
        all_in_names = all_in_names + (partition_name,)

    def _body(*args):
        operands = list(args)
        if partition_name is not None:
            operands.append(b2j.partition_id_tensor())
        outs = b2j._bass_exec_p.bind(
            *operands,
            out_avals=tuple(out_avals),
            in_names=all_in_names,
            out_names=tuple(out_names),
            lowering_input_output_aliases=(),
            sim_require_finite=True,
            sim_require_nnan=True,
            nc=nc,
        )
        return tuple(outs)

    devices = jax.devices()[:N_CORES]
    mesh = Mesh(np.asarray(devices), ("core",))
    in_specs = (PartitionSpec("core"),) * (n_params + n_outs)
    out_specs = (PartitionSpec("core"),) * n_outs
    donate = tuple(range(n_params, n_params + n_outs))
    sharded = jax.jit(
        shard_map(
            _body, mesh=mesh, in_specs=in_specs, out_specs=out_specs,
            check_rep=False,
        ),
        donate_argnums=donate,
        keep_unused=True,
    )

    from jax.sharding import NamedSharding

    zeros_fns = [
        jax.jit(
            lambda s=shape, d=dtype: jax.numpy.zeros(
                (N_CORES * s[0], *s[1:]), d
            ),
            out_shardings=NamedSharding(mesh, PartitionSpec("core")),
        )
        for shape, dtype in zero_shapes
    ]

    def run(named_inputs: dict):
        """named_inputs: name -> concatenated (N_CORES*dim0, ...) array."""
        ins = [named_inputs[n] for n in in_names]
        zeros = [f() for f in zeros_fns]
        outs = sharded(*ins, *zeros)
        return dict(zip(out_names, outs))

    run._parts = {
        "sharded": sharded,
        "in_names": in_names,
        "out_names": out_names,
        "mesh": mesh,
        "zeros_fns": zeros_fns,
    }
    return run


def _get_runner():
    global _RUNNER
    if _RUNNER is None:
        _RUNNER = _build_runner()
    return _RUNNER


def _prep_inputs(y, A, b):
    f16 = _np_f16()
    A64 = A.astype(np.float64)
    W = np.linalg.solve(A64 @ A64.T, A64)  # (M, N)
    # chunks on the PE-accumulate path (odd d) are negated on upload
    sign = np.repeat(np.where(np.arange(KC) % 2 == 1, -1.0, 1.0), 128)
    W_mixed = (W * sign[None, :]).astype(f16)
    # at[p, k, m] = A[m, k*128+p]: stage-1 stationary chunks, contiguous rows
    at_p = np.ascontiguousarray(
        A.reshape(M, KC, 128).transpose(2, 1, 0)
    ).astype(f16)
    f8 = np.dtype(mybir.dt.np(F8))
    y16 = y.astype(f16)
    b16 = b.astype(f8)
    # concat-over-cores layouts expected by the shard_map runner
    yt_cat = np.ascontiguousarray(
        y16.reshape(N_CORES, BC, N).transpose(0, 2, 1)
    ).reshape(N_CORES * N, BC)
    bt_cat = np.ascontiguousarray(
        b16.reshape(N_CORES, BC, M).transpose(0, 2, 1)
    ).reshape(N_CORES * M, BC)
    at_cat = np.broadcast_to(at_p, (N_CORES, 128, KC, M)).reshape(
        N_CORES * 128, KC, M
    )
    w_cat = np.broadcast_to(W_mixed, (N_CORES, M, N)).reshape(N_CORES * M, N)
    return {"yt": yt_cat, "bt": bt_cat, "at": at_cat, "w": w_cat}


def _unpack_output(out_cat: np.ndarray) -> np.ndarray:
    return (
        np.asarray(out_cat)
        .reshape(N_CORES, N, BC)
        .transpose(0, 2, 1)
        .astype(np.float32)
        .reshape(BATCH, N)
    )


def kernel(y: np.ndarray, A: np.ndarray, b: np.ndarray) -> np.ndarray:
    y = np.ascontiguousarray(np.asarray(y, dtype=np.float32))
    A = np.ascontiguousarray(np.asarray(A, dtype=np.float32))
    b = np.ascontiguousarray(np.asarray(b, dtype=np.float32))
    assert y.shape == (BATCH, N) and A.shape == (M, N) and b.shape == (BATCH, M)

    named = _prep_inputs(y, A, b)
    try:
        run = _get_runner()
        out = run(named)["out"]
        return _unpack_output(out)
    except Exception:
        # Fallback: slower but uses only the public SPMD entry point.
        in_maps = [
            {
                k: np.ascontiguousarray(
                    v.reshape(N_CORES, v.shape[0] // N_CORES, *v.shape[1:])[i]
                )
                for k, v in named.items()
            }
            for i in range(N_CORES)
        ]
        res = run_bass_kernel_spmd(_get_nc(), in_maps, list(range(N_CORES)))
        x = np.empty((BATCH, N), dtype=np.float32)
        for i in range(N_CORES):
            x[i * BC:(i + 1) * BC, :] = (
                np.asarray(res.results[i]["out"]).T.astype(np.float32)
            )
        return x
